# revision 1
# baseline (speedup 1.0000x reference)
"""BiAttention Trainium2 kernel (8 NeuronCores, batch-parallel).

Problem (per batch element b, 8 of them -> one per core):
    A_proj = A @ W_A + b_A            [2048, 64]
    B_proj = B @ W_B + b_B            [2048, 64]
    S      = A_proj @ B_proj^T        [2048, 2048]
    A_star = softmax(S, axis=-1) @ B  [2048, 768]
    B_star = softmax(S, axis=0)^T @ A [2048, 768]

Key algebra used on-device (S is small: |S| < ~30, so exp(S) is safe in
f32/bf16 without max-subtraction):
    E = exp(S)
    A_star = diag(1/rowsum(E)) . (E @ B)
    B_star = diag(1/colsum(E)) . (E^T @ A)
rowsum/colsum are obtained for free by augmenting the moving operands
with a ones-column (E @ [B | 1] gives the row sums in the last column).

E is never materialized in full: score panels are recomputed per
512-wide output stripe (K=64 contraction - cheap) directly from the
projections, exp'd into bf16 packs, and immediately consumed as the
stationary operand of the big matmuls.

v3 pipeline notes:
  - Input tiles are cast-loaded f32->bf16 straight into persistent
    [128, 2, 769] staging tiles whose last column is memset to 1.0, so
    the staging tiles ARE the augmented moving operands (no copies).
  - Load order B0,B1,W,A0,A1,B2..B7,A2..A7: the first output stripe
    (direction A, stripe 0) needs all of B but only the first quarter
    of A, so its pack/accum matmuls stream behind the B load.
  - The whole first work item (transposes, projection chunks, score
    packs, ii=0 accumulation) is emitted interleaved in DMA-arrival
    order, so the PE has continuous work from ~10us on (keeps the HAM
    clock gate at 8/8) instead of idling until the full lead-in ends.
"""

import sys

if "/opt/trn_rl_repo" not in sys.path:
    sys.path.insert(0, "/opt/trn_rl_repo")

import numpy as np

import concourse.bass as bass
import concourse.mybir as mybir
import concourse.tile as tile
from concourse import bacc
from concourse.bass import ts
from concourse.bass_utils import run_bass_kernel_spmd
F32 = mybir.dt.float32
BF16 = mybir.dt.bfloat16
AF = mybir.ActivationFunctionType

L = 2048          # sequence length (both La and Lb)
D = 768           # model dim
H = 64            # projection dim
NT = L // 128     # 16 row/col tiles of 128
KD = D // 128     # 6 contraction tiles for the projections
NSUP = L // 512   # 4 supers (512-wide output stripes)
DP = D + 1        # moving operand width with the ones column

N_CORES = 8

_CACHE = {}

import ml_dtypes

_IDENT = np.eye(128, dtype=ml_dtypes.bfloat16)


def _build():
    nc = bacc.Bacc("TRN2", target_bir_lowering=False, debug=False,
                   num_devices=N_CORES)
    A_d = nc.dram_tensor("A", [L, D], F32, kind="ExternalInput").ap()
    B_d = nc.dram_tensor("B", [L, D], F32, kind="ExternalInput").ap()
    WA_d = nc.dram_tensor("W_A", [D, H], F32, kind="ExternalInput").ap()
    WB_d = nc.dram_tensor("W_B", [D, H], F32, kind="ExternalInput").ap()
    bA_d = nc.dram_tensor("b_A", [H, 1], F32, kind="ExternalInput").ap()
    bB_d = nc.dram_tensor("b_B", [H, 1], F32, kind="ExternalInput").ap()
    ID_d = nc.dram_tensor("IDENT", [128, 128], BF16, kind="ExternalInput").ap()
    AS_d = nc.dram_tensor("A_star", [L, D], F32, kind="ExternalOutput").ap()
    BS_d = nc.dram_tensor("B_star", [L, D], F32, kind="ExternalOutput").ap()

    with tile.TileContext(nc) as tc:
        with (
            tc.tile_pool(name="mov", bufs=1) as pmov,
            tc.tile_pool(name="proj", bufs=1) as pproj,
            tc.tile_pool(name="pack", bufs=2) as ppack,
            tc.tile_pool(name="outp", bufs=4) as pout,
            tc.tile_pool(name="psum", bufs=2, space="PSUM") as pps,
        ):
            # identity for TensorE-based transposition (loaded from DRAM)
            ident = pmov.tile([128, 128], BF16, tag="ident", name="ident")

            dram = {"A": A_d, "B": B_d}
            stg = {"A": [], "B": []}   # persistent [128, 2, DP] units
            mts = {}
            projT = {}
            w_sb = {}
            b_sb = {}
            for side in ("A", "B"):
                for u in range(NT // 2):
                    stg[side].append(pmov.tile(
                        [128, 2, DP], BF16, tag=f"stg{side}{u}",
                        name=f"stg{side}{u}"))
                mts[side] = pmov.tile([128, NT * KD, 128], BF16,
                                      tag=f"t{side}", name=f"{side}_T")
                # rows 0:64 written by proj activation; rows 64:128 get a
                # duplicate (SBUF->SBUF DMA) so K=64 score matmuls can be
                # row-packed two-at-a-time with tile_position (0,0)/(64,0)
                projT[side] = pproj.tile([128, L], BF16, tag=f"p{side}",
                                         name=f"{side}_projT")

            def kick_load(side, u):
                # one casting DMA per 2-tile unit: f32 DRAM -> bf16 SBUF
                # (SWDGE), writing the data columns of the persistent
                # staging tile (col D stays the memset ones column)
                nc.gpsimd.dma_start(
                    out=stg[side][u][:, :, 0:D],
                    in_=dram[side][u * 256:(u + 1) * 256, :].rearrange(
                        "(t p) d -> p t d", p=128
                    ),
                )

            def kick_weights(side, W_dram):
                wb = pmov.tile([128, KD, H], BF16, tag=f"w{side}",
                               name=f"w{side}b")
                nc.gpsimd.dma_start(
                    out=wb, in_=W_dram.rearrange("(k p) h -> p k h", p=128)
                )
                w_sb[side] = wb

            def xbar_unit(side, u):
                # DMA xbar transpose (sync HWDGE queue): one call per
                # 128-row tile turns [128, 768] into six transposed
                # [128, 128] blocks laid out exactly as mts wants
                for t in range(2):
                    i = 2 * u + t
                    nc.sync.dma_start(
                        out=mts[side][:, i * KD:(i + 1) * KD, :],
                        in_=stg[side][u][:, t, 0:D],
                        transpose=True,
                    )

            def transpose_unit(side, u):
                # transpose the unit's 12 blocks on TensorE via identity;
                # psum drained in 3-block halves by the (otherwise idle)
                # vector engine so the psum ring recycles fast
                for t in range(2):
                    i = 2 * u + t
                    for half in range(2):
                        ps = pps.tile([128, 512], F32, tag="spack", bufs=4,
                                      name=f"pstr{side}{i}h{half}")
                        for j3 in range(3):
                            j = half * 3 + j3
                            nc.tensor.matmul(ps[:, ts(j3, 128)],
                                             stg[side][u][:, t, ts(j, 128)],
                                             ident, start=True, stop=True)
                        nc.vector.tensor_copy(
                            out=mts[side][:, i * KD + half * 3:
                                          i * KD + half * 3 + 3, :],
                            in_=ps[:, 0:384],
                        )

            def proj_chunk(side, c):
                # projT[h, 512c:512(c+1)] = sum_d W[d,h] M^T[d, s-chunk]
                mtv = mts[side].rearrange("p (i j) q -> p i j q", j=KD)
                ps = pps.tile([128, 512], F32, tag="spack", bufs=4,
                              name=f"psproj{side}{c}")
                for k in range(KD):
                    nc.tensor.matmul(
                        ps[:H, 0:512],
                        w_sb[side][:, k, :],
                        mtv[:, 4 * c:4 * c + 4, k, :],
                        start=(k == 0), stop=(k == KD - 1),
                    )
                nc.scalar.activation(
                    out=projT[side][0:H, ts(c, 512)], in_=ps[:H, 0:512],
                    func=AF.Identity, bias=b_sb[side], scale=1.0,
                )
                # duplicate into partitions 64:128 for row-packed S matmuls
                # (sync queue: keeps the dup off the scalar stream, where
                # it would queue behind exps and stall pack LDWs)
                nc.sync.dma_start(out=projT[side][64:128, ts(c, 512)],
                                  in_=projT[side][0:H, ts(c, 512)])

            # ---- kicks: biases (scalar), ident (sync), inputs+weights
            # (gpsimd, in the order the pipeline consumes them) ----
            for side, b_dram in (("B", bB_d), ("A", bA_d)):
                bt = pmov.tile([H, 1], F32, tag=f"b{side}", name=f"b{side}sb")
                nc.scalar.dma_start(out=bt, in_=b_dram)
                b_sb[side] = bt
            nc.sync.dma_start(out=ident, in_=ID_d)
            # interleaved load order: B and A units alternate after the
            # prefix, so transpose work streams continuously behind the
            # DMA (the PE never idles long enough to drop the HAM clock)
            kick_load("B", 0)
            kick_load("B", 1)
            kick_weights("B", WB_d)
            kick_weights("A", WA_d)
            kick_load("A", 0)
            kick_load("A", 1)
            for u in range(2, 8):
                kick_load("B", u)
                kick_load("A", u)

            # ones columns (vector; disjoint from the load columns)
            for side in ("B", "A"):
                for u in range(NT // 2):
                    nc.vector.memset(stg[side][u][:, :, D:DP], 1.0)

            # HAM warmup + fillers: dependency-free matmuls into a
            # dedicated psum bank keep the PE's activity window busy while
            # it would otherwise wait on DMA (an idle 3.4us window halves
            # the PE clock, which costs far more than the filler work)
            wmov = pmov.tile([128, 512], BF16, tag="warmmv", name="warmmv")
            nc.vector.memset(wmov, 0.125)
            wps = pps.tile([128, 1024], F32, tag="accum", name="warmps")
            for _ in range(28):
                nc.tensor.matmul(wps[:, 0:512], ident, wmov,
                                 start=True, stop=True)

            # ---- main: per 512-wide output stripe ----
            # dirn "A": produce A_star rows; panels are E'[t, s-stripe]
            #   (lhsT = B_projT tiles, rhs = A_projT stripe), moving = B stg
            # dirn "B": produce B_star rows; panels are E[s, t-stripe]
            #   (lhsT = A_projT tiles, rhs = B_projT stripe), moving = A stg
            work = [("A", u) for u in range(NSUP)] + \
                   [("B", u) for u in range(NSUP)]
            spec = {
                "A": (projT["B"], projT["A"], stg["B"], AS_d),
                "B": (projT["A"], projT["B"], stg["A"], BS_d),
            }
            packs = {}

            def pack_pair(w, jp):
                dirn, u = w
                pT_l, pT_r, _, _ = spec[dirn]
                pkt = ppack.tile([128, 1024], BF16, tag="pack", bufs=26,
                                 name=f"pk{dirn}{u}{jp}")
                for h2 in range(2):
                    # row-packed pair: K=64 matmuls in rows 0:64 / 64:128,
                    # each into its own 1-bank psum with its own exp so
                    # the ring recycles per-half
                    j = jp * 2 + h2
                    base = h2 * 64
                    ps = pps.tile([128, 512], F32, tag="spack", bufs=4,
                                  name=f"pss{dirn}{u}{jp}h{h2}")
                    nc.tensor.matmul(
                        ps,
                        pT_l[base:base + H, ts(j, 128)],
                        pT_r[base:base + H, ts(u, 512)],
                        start=True, stop=True,
                        tile_position=(base, 0),
                    )
                    nc.scalar.activation(out=pkt[:, ts(h2, 512)], in_=ps,
                                         func=AF.Exp)
                packs.setdefault(w, []).append(pkt)

            def emit_pack(w):
                for jp in range(NT // 2):
                    pack_pair(w, jp)

            def accum_jpair(w, pa, jp, ii=0):
                dirn, u = w
                _, _, mv, _ = spec[dirn]
                pks = packs[w]
                for j in (2 * jp, 2 * jp + 1):
                    lhs = pks[j // 2][:, (j % 2) * 512 + ii * 128:
                                      (j % 2) * 512 + ii * 128 + 128]
                    mvt = mv[j // 2]
                    # short mm first: the trailing 512-col mm covers the
                    # next pair's LDWEIGHTS pull-ahead window
                    nc.tensor.matmul(
                        pa[:, 512:DP], lhs, mvt[:, j % 2, 512:DP],
                        start=(j == 0), stop=(j == NT - 1),
                    )
                    nc.tensor.matmul(
                        pa[:, 0:512], lhs, mvt[:, j % 2, 0:512],
                        start=(j == 0), stop=(j == NT - 1),
                    )

            def finish_tile(w, ii, pa):
                dirn, u = w
                _, _, _, out_d = spec[dirn]
                rinv = pout.tile([128, 1], F32, tag="rinv",
                                 name=f"ri{dirn}{u}{ii}")
                nc.vector.reciprocal(out=rinv, in_=pa[:, D:DP])
                ot = pout.tile([128, D], F32, tag="ot",
                               name=f"ot{dirn}{u}{ii}")
                nc.vector.tensor_scalar_mul(ot, pa[:, 0:D], rinv)
                nc.sync.dma_start(out=out_d[ts(u * 4 + ii, 128), :], in_=ot)

            def emit_accum_tile(w, ii):
                dirn, u = w
                pa = pps.tile([128, 1024], F32, tag="accum",
                              name=f"pa{dirn}{u}{ii}")
                for jp in range(NT // 2):
                    accum_jpair(w, pa, jp, ii)
                finish_tile(w, ii, pa)

            # ---- work item 0: everything interleaved in DMA-arrival
            # order (transposes, proj chunks, pack pairs, ii=0 accums),
            # per-unit granularity so PE waits stay short (HAM clock) ----
            # two stripes' packs and ii=0 accumulations ride the load
            # window, so the PE backlog never drains (HAM clock stays 8/8)
            w0, w1 = work[0], work[1]
            transpose_unit("B", 0)
            transpose_unit("B", 1)
            proj_chunk("B", 0)
            transpose_unit("A", 0)
            transpose_unit("A", 1)
            proj_chunk("A", 0)
            pa0 = pps.tile([128, 1024], F32, tag="accum", name="paA00")
            pa1 = pps.tile([128, 1024], F32, tag="accum", name="paA10")
            pack_pair(w0, 0)
            pack_pair(w0, 1)
            accum_jpair(w0, pa0, 0)
            for u in range(2, 8):
                transpose_unit("B", u)
                if u % 2 == 1:
                    proj_chunk("B", u // 2)
                    pack_pair(w0, u - 1)
                    pack_pair(w0, u)
                accum_jpair(w0, pa0, u - 1)
                transpose_unit("A", u)
                if u % 2 == 1:
                    proj_chunk("A", u // 2)
                    pack_pair(w1, u - 3)
                    pack_pair(w1, u - 2)
                if u >= 4:
                    accum_jpair(w1, pa1, u - 4)
            accum_jpair(w0, pa0, 7)
            finish_tile(w0, 0, pa0)
            pack_pair(w1, 6)
            pack_pair(w1, 7)
            for jp in range(4, 8):
                accum_jpair(w1, pa1, jp)
            finish_tile(w1, 0, pa1)

            # remaining output tiles of (A,0)/(A,1), with packs for the
            # following stripes woven two-per-tile into the accum stream
            pair_cursor = {}

            def weave_packs(wn, k=2):
                c = pair_cursor.setdefault(wn, 0)
                while c < NT // 2 and k > 0:
                    pack_pair(wn, c)
                    c += 1
                    k -= 1
                pair_cursor[wn] = c

            w2, w3 = work[2], work[3]
            for ii in range(1, 4):
                emit_accum_tile(w0, ii)
                weave_packs(w2)
                emit_accum_tile(w1, ii)
                weave_packs(w2) if pair_cursor.get(w2, 0) < 8 else \
                    weave_packs(w3)
            packs.pop(w0, None)

            # ---- remaining work items ----
            for idx in range(2, len(work)):
                w = work[idx]
                wn = work[idx + 1] if idx + 1 < len(work) else None
                for ii in range(4):
                    emit_accum_tile(w, ii)
                    if wn is not None:
                        weave_packs(wn)
                packs.pop(work[idx - 1], None)

    nc.compile()
    return nc


def _get_nc():
    if "nc" not in _CACHE:
        _CACHE["nc"] = _build()
    return _CACHE["nc"]


def _run(inputs, trace=False):
    nc = _get_nc()
    A = np.ascontiguousarray(np.asarray(inputs["A"], dtype=np.float32))
    B = np.ascontiguousarray(np.asarray(inputs["B"], dtype=np.float32))
    W_A = np.ascontiguousarray(np.asarray(inputs["W_A"], dtype=np.float32))
    W_B = np.ascontiguousarray(np.asarray(inputs["W_B"], dtype=np.float32))
    b_A = np.asarray(inputs["b_A"], dtype=np.float32).reshape(H, 1)
    b_B = np.asarray(inputs["b_B"], dtype=np.float32).reshape(H, 1)
    in_maps = [
        {
            "A": A[c], "B": B[c],
            "W_A": W_A, "W_B": W_B,
            "b_A": b_A, "b_B": b_B,
            "IDENT": _IDENT,
        }
        for c in range(N_CORES)
    ]
    res = run_bass_kernel_spmd(nc, in_maps, list(range(N_CORES)), trace=trace)
    A_star = np.stack([res.results[c]["A_star"] for c in range(N_CORES)])
    B_star = np.stack([res.results[c]["B_star"] for c in range(N_CORES)])
    return A_star, B_star, res


def kernel(**inputs):
    A_star, B_star, _ = _run(inputs)
    return A_star, B_star

